# revision 8
# baseline (speedup 1.0000x reference)
"""AttentionConv (sparse local attention, 7x7 window, per-channel softmax)
Trainium2 Bass kernel, SPMD across 8 NeuronCores.

Sharding: core i handles batch b = i//2 and channel half cg = i%2
(channels are independent through the whole op: 1x1 convs produce each
output channel from all input channels, and the softmax is per-channel
over the 7x7 window).

The relative-position bias for channels [0,128) is rel_h[u] (window row)
and for channels [128,256) is rel_w[v] (window col). To keep one SPMD
program for all cores, cg=1 cores receive spatially TRANSPOSED x (H<->W)
and their output is transposed back on the host; under that transpose
rel_w becomes a window-row bias, identical in structure to cg=0.

Per-core pipeline (fp16 score path, bf16 value path, f32 accumulate):
  1. PE GEMMs: q,k,v = W @ x in fp16. K=256 contraction in 2 chunks,
     N chunks of 512 (one PSUM bank each).
  2. k,v scattered into zero-padded 38x40 planes; each plane stored
     twice (interior at col 3 and col 2) so windowed reads for even AND
     odd window-cols are 4-byte aligned -> DVE 16-bit 2x perf mode.
  3. 7 bias-added copies of each padded k plane, trimmed to the 32 rows
     window-row u actually reads (tensor_scalar 4x mode).
     (scalar_tensor_tensor would fuse this into the mul but measures 1x
     on HW -- no 2x uop -- so pre-biased planes + plain TT 2x win.)
  4. main loop over window col v (7 iters), u-dim split in (4,3) halves,
     diagonal access patterns covering all u at once:
       s = q * kb[window]  fp16     (DVE TT, 2x)
       e = exp(s) -> bf16           (ScalarE ACT, unnormalized)
       m = e * v[window]  bf16      (DVE TT, 2x)
       num += I @ m ; den += I @ e  (TensorE identity matmuls, PSUM f32)
     DVE program order is software-pipelined one v ahead: the score
     muls for v+1 are emitted before the m-muls of v, so the ACT exp
     latency never stalls the DVE.
  5. tail: den matmuls for the last v emitted straight after its exp
     (before the m muls) so the reciprocal unblocks early; then
     out = num * reciprocal_approx_fast(den); split-queue DMA out.
"""

import os

import numpy as np
import ml_dtypes

K = 7
PAD = 3
H = W = 32
HW = H * W
B = 4
C = 256
RS = 40          # padded plane row stride (elements); even => alignment
PR = H + 2 * PAD  # 38 padded rows
PW = PR * RS     # padded plane size per partition
N_CORES = 8

_NC_CACHE = {}


def _build_nc():
    import concourse.bass as bass
    import concourse.tile as tile
    from concourse import mybir, bacc

    bf16 = mybir.dt.bfloat16
    f16 = mybir.dt.float16
    f32 = mybir.dt.float32

    nc = bacc.Bacc(None)
    x_ext = nc.dram_tensor("x", [2, 128, HW], f16, kind="ExternalInput")
    w_ext = nc.dram_tensor("w", [3, 2, 128, 128], f16, kind="ExternalInput")
    b_ext = nc.dram_tensor("bias", [128, K], f32, kind="ExternalInput")
    i_ext = nc.dram_tensor("ident", [128, 128], bf16, kind="ExternalInput")
    o_ext = nc.dram_tensor("out", [128, HW], f32, kind="ExternalOutput")

    with tile.TileContext(nc) as tc:
        with (
            tc.tile_pool(name="consts", bufs=1) as consts,
            tc.tile_pool(name="kv", bufs=1) as kv,
            tc.tile_pool(name="fin", bufs=1) as fin,
            tc.tile_pool(name="psa", bufs=1, space="PSUM") as psa,
            tc.tile_pool(name="gt", bufs=1) as gt,
            tc.tile_pool(name="psg", bufs=4, space="PSUM") as psg,
            tc.tile_pool(name="sp", bufs=3) as sp,
            tc.tile_pool(name="ep", bufs=3) as ep,
            tc.tile_pool(name="mp", bufs=3) as mp,
        ):
            xsb = gt.tile([128, 2, HW], f16)
            wsb = gt.tile([128, 3, 2, 128], f16)
            bsb = consts.tile([128, K], f32)
            isb = consts.tile([128, 128], bf16)
            # q plane (read via a 7-way broadcast AP in the main loop)
            qsb = kv.tile([128, H, W], f16)
            # padded k/v planes; E holds interior at col 3 (for even v
            # window reads), O at col 2 (odd v reads at offset v-1).
            kpE = gt.tile([128, PR, RS], f16)
            kpO = gt.tile([128, PR, RS], f16)
            vpE = kv.tile([128, PR, RS], bf16)
            vpO = kv.tile([128, PR, RS], bf16)
            kbE = kv.tile([128, K, PR, RS], f16)
            kbO = kv.tile([128, K, PR, RS], f16)

            # DMAs: k-GEMM inputs first on both fast queues; x split so
            # the first k matmuls' data lands early. ident first on
            # gpsimd (gates the PE warm-up matmuls).
            nc.gpsimd.dma_start(out=isb[:], in_=i_ext[:])
            nc.sync.dma_start(out=wsb[:, 1, 0, :], in_=w_ext[1, 0])
            nc.scalar.dma_start(out=wsb[:, 1, 1, :], in_=w_ext[1, 1])
            nc.sync.dma_start(out=xsb[:, 0, 0:512], in_=x_ext[0][:, 0:512])
            nc.scalar.dma_start(out=xsb[:, 1, 0:512], in_=x_ext[1][:, 0:512])
            nc.sync.dma_start(out=xsb[:, 0, 512:HW], in_=x_ext[0][:, 512:HW])
            nc.scalar.dma_start(out=xsb[:, 1, 512:HW], in_=x_ext[1][:, 512:HW])
            nc.gpsimd.dma_start(out=bsb[:], in_=b_ext[:])
            nc.sync.dma_start(out=wsb[:, 0, 0, :], in_=w_ext[0, 0])
            nc.scalar.dma_start(out=wsb[:, 0, 1, :], in_=w_ext[0, 1])
            nc.sync.dma_start(out=wsb[:, 2, 0, :], in_=w_ext[2, 0])
            nc.scalar.dma_start(out=wsb[:, 2, 1, :], in_=w_ext[2, 1])

            # Plane zero-init split across gpsimd (k) and scalar (v).
            nc.gpsimd.memset(kpE[:], 0.0)
            nc.gpsimd.memset(kpO[:], 0.0)
            nc.scalar.memzero(vpE[:])
            nc.scalar.memzero(vpO[:])

            nps = psa.tile([128, HW], f32)
            dps = psa.tile([128, HW], f32)

            # PE pipeline/HAM warm-up: dummy matmuls into nps, whose
            # content is discarded when the first start=True accumulation
            # clears has_written.
            for _ in range(3):
                nc.tensor.matmul(nps[:, 0:128], isb[:], isb[:],
                                 start=True, stop=True, skip_group_check=True)

            # ---- GEMMs: wi 0=q, 1=k, 2=v; N chunks of 512 px (16 rows)
            # k first: it gates the longest pre-loop chain (bias copies).
            for wi in (1, 0, 2):
                for ch in range(2):
                    ps = psg.tile([128, 16, 32], f32)
                    for ci in range(2):
                        nc.tensor.matmul(
                            ps[:],
                            wsb[:, wi, ci, :],
                            xsb[:, ci, ch * 512:(ch + 1) * 512],
                            start=(ci == 0),
                            stop=(ci == 1),
                        )
                    r0 = PAD + 16 * ch
                    if wi == 0:
                        nc.scalar.copy(qsb[:, 16 * ch:16 * ch + 16, :], ps[:])
                    elif wi == 1:
                        nc.vector.tensor_copy(kpE[:, r0:r0 + 16, 3:35], ps[:])
                        nc.scalar.copy(kpO[:, r0:r0 + 16, 2:34], ps[:])
                    else:
                        nc.scalar.copy(vpE[:, r0:r0 + 16, 3:35], ps[:])
                        nc.scalar.copy(vpO[:, r0:r0 + 16, 2:34], ps[:])

            # biased k copies: kb*[u] = kp* + bias[:, u] (per-partition,
            # tensor_scalar 4x), trimmed to rows u..u+31 -- the only rows
            # the window for row u reads.
            def bias_copies(kb, kp):
                for u in range(K):
                    nc.vector.tensor_scalar_add(
                        kb[:, u, u:u + H, :], kp[:, u:u + H, :],
                        bsb[:, u:u + 1])

            def window_ap(t, base_off, u0, nu, u_step):
                full = t[:]
                return bass.AP(
                    tensor=full.tensor,
                    offset=full.offset + base_off + u0 * u_step,
                    ap=[full.ap[0], [u_step, nu], [RS, H], [1, W]],
                )

            def q_bcast(nu):
                full = qsb[:]
                return bass.AP(
                    tensor=full.tensor,
                    offset=full.offset,
                    ap=[full.ap[0], [0, nu], [W, H], [1, W]],
                )

            HALVES = ((0, 4), (4, 3))
            LAST = K - 1

            def planes(v):
                if v & 1:
                    return kbO, vpO, v - 1
                return kbE, vpE, v

            s_tiles = {}
            e_tiles = {}

            def emit_s(v, hi):
                u0, nu = HALVES[hi]
                kb, _, off = planes(v)
                s = sp.tile([128, nu, H, W], f16, tag=f"s{hi}")
                nc.vector.tensor_mul(
                    s[:], q_bcast(nu), window_ap(kb, off, u0, nu, PW + RS))
                e = ep.tile([128, nu, H, W], bf16, tag=f"e{hi}")
                nc.scalar.activation(
                    e[:], s[:], mybir.ActivationFunctionType.Exp)
                s_tiles[(v, hi)] = s
                e_tiles[(v, hi)] = e

            def emit_den(v, hi):
                u0, nu = HALVES[hi]
                e = e_tiles[(v, hi)]
                first = v == 0
                last = v == LAST
                for ch in range(2):
                    csl = slice(ch * 512, (ch + 1) * 512)
                    rsl = slice(16 * ch, 16 * ch + 16)
                    for du in range(nu):
                        u = u0 + du
                        nc.tensor.matmul(
                            dps[:, csl], isb[:], e[:, du, rsl, :],
                            start=(first and u == 0),
                            stop=(last and u == K - 1),
                            skip_group_check=True,
                        )

            def emit_m(v, hi, with_den=True):
                u0, nu = HALVES[hi]
                _, vp, off = planes(v)
                e = e_tiles[(v, hi)]
                m = mp.tile([128, nu, H, W], bf16, tag=f"m{hi}")
                nc.vector.tensor_mul(
                    m[:], e[:], window_ap(vp, off, u0, nu, RS))
                first = v == 0
                last = v == LAST
                for du in range(nu):
                    u = u0 + du
                    for ch in range(2):
                        csl = slice(ch * 512, (ch + 1) * 512)
                        rsl = slice(16 * ch, 16 * ch + 16)
                        if with_den:
                            nc.tensor.matmul(
                                dps[:, csl], isb[:], e[:, du, rsl, :],
                                start=(first and u == 0),
                                stop=(last and u == K - 1),
                                skip_group_check=True,
                            )
                        nc.tensor.matmul(
                            nps[:, csl], isb[:], m[:, du, rsl, :],
                            start=(first and u == 0),
                            stop=(last and u == K - 1),
                            skip_group_check=True,
                        )
                return m

            # ---- software-pipelined main loop over window col v.
            bias_copies(kbE, kpE)
            emit_s(0, 0)
            emit_s(0, 1)
            bias_copies(kbO, kpO)
            m_tail = {}
            for v in range(K):
                if v < LAST:
                    # next-v scores fill the DVE while ACT runs exp(v)
                    emit_s(v + 1, 0)
                    emit_m(v, 0)
                    emit_s(v + 1, 1)
                    emit_m(v, 1)
                else:
                    # tail: den accumulation straight after exp (before
                    # the m muls) so the reciprocal unblocks early; num
                    # matmuls sorted by chunk for the same reason.
                    emit_den(v, 0)
                    emit_den(v, 1)
                    for hi in range(2):
                        u0, nu = HALVES[hi]
                        _, vp, off = planes(v)
                        m = mp.tile([128, nu, H, W], bf16, tag=f"m{hi}")
                        nc.vector.tensor_mul(
                            m[:], e_tiles[(v, hi)],
                            window_ap(vp, off, u0, nu, RS))
                        m_tail[hi] = m
                    for ch in range(2):
                        csl = slice(ch * 512, (ch + 1) * 512)
                        rsl = slice(16 * ch, 16 * ch + 16)
                        for hi in range(2):
                            u0, nu = HALVES[hi]
                            for du in range(nu):
                                u = u0 + du
                                nc.tensor.matmul(
                                    nps[:, csl], isb[:],
                                    m_tail[hi][:, du, rsl, :],
                                    start=False,
                                    stop=(u == K - 1),
                                    skip_group_check=True,
                                )

            # ---- normalize and store: both reciprocals first (they
            # overlap the PE's final num matmuls), then multiply + DMA
            # per half on separate queues.
            rden = fin.tile([128, HW], f32)
            outsb = fin.tile([128, HW], f32)
            for ch in (0, 1):
                csl = slice(ch * 512, (ch + 1) * 512)
                nc.vector.reciprocal_approx_fast(
                    out=rden[:, csl], in_=dps[:, csl])
            for ch, eng in ((0, nc.sync), (1, nc.scalar)):
                csl = slice(ch * 512, (ch + 1) * 512)
                nc.vector.tensor_mul(
                    outsb[:, csl], nps[:, csl], rden[:, csl])
                eng.dma_start(out=o_ext[:, csl], in_=outsb[:, csl])

    nc.finalize()
    return nc


def _get_nc():
    if "nc" not in _NC_CACHE:
        _NC_CACHE["nc"] = _build_nc()
    return _NC_CACHE["nc"]


def _prep_in_maps(x, wq, wk, wv, rel_h, rel_w):
    bf = ml_dtypes.bfloat16
    ident = np.eye(128, dtype=bf)
    in_maps = []
    for core in range(N_CORES):
        b, cg = divmod(core, 2)
        xb = np.asarray(x[b], dtype=np.float32)
        if cg == 1:
            xb = xb.transpose(0, 2, 1)
        xb = np.ascontiguousarray(xb).reshape(2, 128, HW).astype(np.float16)
        rows = slice(cg * 128, (cg + 1) * 128)
        wt = np.stack([np.asarray(wq)[rows], np.asarray(wk)[rows],
                       np.asarray(wv)[rows]])          # [3, 128, 256]
        wt = np.ascontiguousarray(
            wt.transpose(0, 2, 1).astype(np.float16)).reshape(
            3, 2, 128, 128)                            # [wi, ci_chunk, ci, co]
        bias = np.ascontiguousarray(
            np.asarray(rel_h if cg == 0 else rel_w, dtype=np.float32))
        in_maps.append({"x": xb, "w": wt, "bias": bias, "ident": ident})
    return in_maps


def _assemble(results):
    out = np.empty((B, C, H, W), np.float32)
    for core in range(N_CORES):
        b, cg = divmod(core, 2)
        o = results[core]["out"].reshape(128, H, W)
        if cg == 1:
            o = o.transpose(0, 2, 1)
        out[b, cg * 128:(cg + 1) * 128] = o
    return out


def run(inputs, trace=False):
    """Returns (output, BassKernelResults)."""
    from concourse import bass_utils

    nc = _get_nc()
    in_maps = _prep_in_maps(**inputs)
    last_err = None
    for _attempt in range(3):
        try:
            res = bass_utils.run_bass_kernel_spmd(
                nc, in_maps, core_ids=list(range(N_CORES)), trace=trace)
            return _assemble(res.results), res
        except Exception as err:  # transient NRT device errors
            last_err = err
    raise last_err


def kernel(x, wq, wk, wv, rel_h, rel_w):
    out, _ = run(
        dict(x=x, wq=wq, wk=wk, wv=wv, rel_h=rel_h, rel_w=rel_w),
        trace=bool(os.environ.get("ATTNCONV_TRACE")),
    )
    return out


# revision 13
# speedup vs baseline: 1.1574x; 1.1574x over previous
"""AttentionConv (sparse local attention, 7x7 window, per-channel softmax)
Trainium2 Bass kernel, SPMD across 8 NeuronCores.

Sharding: core i handles batch b = i//2 and channel half cg = i%2
(channels are independent through the whole op: 1x1 convs produce each
output channel from all input channels, and the softmax is per-channel
over the 7x7 window).

The relative-position bias for channels [0,128) is rel_h[u] (window row)
and for channels [128,256) is rel_w[v] (window col). To keep one SPMD
program for all cores, cg=1 cores receive spatially TRANSPOSED x (H<->W)
and their output is transposed back on the host; under that transpose
rel_w becomes a window-row bias, identical in structure to cg=0.

Per-core pipeline (fp16 score path, bf16 value path, f32 accumulate):
  1. PE GEMMs: q,k,v = W @ x in fp16 (negligible rounding vs fp32 for
     this data, half the DMA bytes, 16-bit matmul speed). K=256
     contraction in 2 chunks, N chunks of 512 (one PSUM bank each).
  2. k,v scattered into zero-padded 38x40 planes; each plane stored
     twice (interior at col 3 and col 2) so windowed reads for even AND
     odd window-cols are 4-byte aligned -> DVE 16-bit 2x perf mode.
  3. 7 bias-added copies of each padded k plane (bias for a fixed
     window-row is a per-partition scalar -> tensor_scalar 4x mode),
     trimmed to the 32 rows each window-row actually reads.
  4. main loop over window col v (7 iters), u-dim split in (4,3) halves
     for pipelining, diagonal access patterns covering all u at once:
       s = q * k_biased[window]  fp16     (DVE TT, 2x)
       e = exp(s) -> bf16                 (ScalarE ACT, unnormalized --
                                           scores are far inside exp's
                                           f32/bf16 range, so no
                                           max-subtraction pass needed)
       m = e * v[window]  bf16            (DVE TT, 2x)
       num += I @ m ; den += I @ e        (TensorE identity matmuls
                                           accumulating in PSUM f32;
                                           the otherwise-idle PE does
                                           all the j-summation work)
  5. out = num * reciprocal_approx_fast(den); split-queue DMA out.
Engine budget per core: DVE ~62us (bottleneck: 2 multiplies per window
element at 2 elem/cyc/lane), ACT ~46us, PE ~52us, ~85us measured total.
NOTE (this session): aggressive software-pipelining of the DVE order
and STT-fused bias both REGRESS on HW (98.6us / 111us): overlapping
DVE+ACT+PE more densely inflates every op 8-25% (SBUF port contention),
and scalar_tensor_tensor has no 2x uop. Keep the baseline schedule;
only local, contention-neutral deltas win.
"""

import os

import numpy as np
import ml_dtypes

K = 7
PAD = 3
H = W = 32
HW = H * W
B = 4
C = 256
RS = 40          # padded plane row stride (elements); even => alignment
PR = H + 2 * PAD  # 38 padded rows
PW = PR * RS     # padded plane size per partition
N_CORES = 8

_NC_CACHE = {}


def _build_nc():
    import concourse.bass as bass
    import concourse.tile as tile
    from concourse import mybir, bacc

    bf16 = mybir.dt.bfloat16
    f16 = mybir.dt.float16
    f32 = mybir.dt.float32

    nc = bacc.Bacc(None)
    x_ext = nc.dram_tensor("x", [2, 128, HW], f16, kind="ExternalInput")
    w_ext = nc.dram_tensor("w", [3, 2, 128, 128], f16, kind="ExternalInput")
    b_ext = nc.dram_tensor("bias", [128, K], f32, kind="ExternalInput")
    i_ext = nc.dram_tensor("ident", [128, 128], bf16, kind="ExternalInput")
    o_ext = nc.dram_tensor("out", [128, HW], f32, kind="ExternalOutput")

    with tile.TileContext(nc) as tc:
        with (
            tc.tile_pool(name="consts", bufs=1) as consts,
            tc.tile_pool(name="kv", bufs=1) as kv,
            tc.tile_pool(name="fin", bufs=1) as fin,
            tc.tile_pool(name="psa", bufs=1, space="PSUM") as psa,
            tc.tile_pool(name="gt", bufs=1) as gt,
            tc.tile_pool(name="psg", bufs=4, space="PSUM") as psg,
            tc.tile_pool(name="sp", bufs=3) as sp,
            tc.tile_pool(name="ep", bufs=3) as ep,
            tc.tile_pool(name="mp", bufs=3) as mp,
        ):
            # DMAs spread across engine queues so they don't serialize.
            xsb = gt.tile([128, 2, HW], f16)
            wsb = gt.tile([128, 3, 2, 128], f16)
            bsb = consts.tile([128, K], f32)
            isb = consts.tile([128, 128], bf16)
            # DMA order tuned for the startup critical path: the first
            # k-GEMM matmul needs x(ci,0:512)+w_k, so those are the first
            # two issue slots on both fast queues; ident stays first on
            # gpsimd (gates the PE warm-ups, which also start the PE
            # frequency ramp).
            nc.gpsimd.dma_start(out=isb[:], in_=i_ext[:])
            nc.sync.dma_start(out=xsb[:, 0, 0:512], in_=x_ext[0][:, 0:512])
            nc.scalar.dma_start(out=xsb[:, 1, 0:512], in_=x_ext[1][:, 0:512])
            nc.sync.dma_start(out=wsb[:, 1, 0, :], in_=w_ext[1, 0])
            nc.scalar.dma_start(out=wsb[:, 1, 1, :], in_=w_ext[1, 1])
            nc.gpsimd.dma_start(out=bsb[:], in_=b_ext[:])
            nc.sync.dma_start(out=xsb[:, 0, 512:HW], in_=x_ext[0][:, 512:HW])
            nc.scalar.dma_start(out=xsb[:, 1, 512:HW], in_=x_ext[1][:, 512:HW])
            nc.sync.dma_start(out=wsb[:, 0, 0, :], in_=w_ext[0, 0])
            nc.scalar.dma_start(out=wsb[:, 0, 1, :], in_=w_ext[0, 1])
            nc.sync.dma_start(out=wsb[:, 2, 0, :], in_=w_ext[2, 0])
            nc.scalar.dma_start(out=wsb[:, 2, 1, :], in_=w_ext[2, 1])

            # q plane (read via a 7-way broadcast AP in the main loop)
            qsb = kv.tile([128, H, W], f16)
            # padded k/v planes; E holds interior at col 3 (for even v
            # window reads), O at col 2 (odd v reads at offset v-1).
            kpE = gt.tile([128, PR, RS], f16)
            kpO = gt.tile([128, PR, RS], f16)
            vpE = kv.tile([128, PR, RS], bf16)
            vpO = kv.tile([128, PR, RS], bf16)
            kbE = kv.tile([128, K, PR, RS], f16)
            kbO = kv.tile([128, K, PR, RS], f16)

            # k-plane zero-init: only the PAD region needs zeros (the
            # interior is overwritten by the GEMM scatter), and the
            # otherwise-idle-at-startup DVE does it off the gpsimd/scalar
            # queues' critical paths. v-planes on the scalar queue after
            # its DMAs.
            for kp, c0 in ((kpE, 3), (kpO, 2)):  # interior at cols c0..c0+31
                nc.vector.memset(kp[:, 0:PAD, :], 0.0)           # top rows
                nc.vector.memset(kp[:, PAD + H:PR, :], 0.0)      # bottom rows
                nc.vector.memset(kp[:, PAD:PAD + H, 0:c0], 0.0)  # left cols
                nc.vector.memset(kp[:, PAD:PAD + H, c0 + H:RS], 0.0)  # right
            nc.scalar.memzero(vpE[:])
            nc.scalar.memzero(vpO[:])

            nps = psa.tile([128, HW], f32)
            dps = psa.tile([128, HW], f32)

            # PE pipeline/HAM warm-up: dummy matmuls into nps, whose
            # content is discarded when the first start=True accumulation
            # clears has_written.
            for _ in range(3):
                nc.tensor.matmul(nps[:, 0:128], isb[:], isb[:],
                                 start=True, stop=True, skip_group_check=True)

            # ---- GEMMs: wi 0=q, 1=k, 2=v; N chunks of 512 px (16 rows)
            # k first: it gates the longest pre-loop chain (bias copies).
            for wi in (1, 0, 2):
                for ch in range(2):
                    ps = psg.tile([128, 16, 32], f32)
                    for ci in range(2):
                        nc.tensor.matmul(
                            ps[:],
                            wsb[:, wi, ci, :],
                            xsb[:, ci, ch * 512:(ch + 1) * 512],
                            start=(ci == 0),
                            stop=(ci == 1),
                        )
                    r0 = PAD + 16 * ch
                    if wi == 0:
                        nc.scalar.copy(qsb[:, 16 * ch:16 * ch + 16, :], ps[:])
                    elif wi == 1:
                        nc.vector.tensor_copy(kpE[:, r0:r0 + 16, 3:35], ps[:])
                        nc.scalar.copy(kpO[:, r0:r0 + 16, 2:34], ps[:])
                    else:
                        nc.scalar.copy(vpE[:, r0:r0 + 16, 3:35], ps[:])
                        nc.scalar.copy(vpO[:, r0:r0 + 16, 2:34], ps[:])

            # biased k copies: kb*[u] = kp* + bias[:, u] (per-partition),
            # trimmed to rows u..u+31 -- the only rows window-row u reads.
            for u in range(K):
                nc.vector.tensor_scalar_add(
                    kbE[:, u, u:u + H, :], kpE[:, u:u + H, :], bsb[:, u:u + 1])
            for u in range(K):
                nc.vector.tensor_scalar_add(
                    kbO[:, u, u:u + H, :], kpO[:, u:u + H, :], bsb[:, u:u + 1])

            def window_ap(t, base_off, u0, nu, u_step):
                full = t[:]
                return bass.AP(
                    tensor=full.tensor,
                    offset=full.offset + base_off + u0 * u_step,
                    ap=[full.ap[0], [u_step, nu], [RS, H], [1, W]],
                )

            def q_bcast(nu):
                full = qsb[:]
                return bass.AP(
                    tensor=full.tensor,
                    offset=full.offset,
                    ap=[full.ap[0], [0, nu], [W, H], [1, W]],
                )

            # u-dim halves for finer DVE->ACT->PE pipelining. Measured
            # optimum: full-width ops 89.8us, quarters 87.2us, (3,4)
            # order 88.2us, this (4,3) split 85.4us. GpSimd offloads of
            # any slab regress (its in-loop TT is 3-5x slower than DVE).
            HALVES = ((0, 4), (4, 3))

            # ---- main loop over window col v
            for v in range(K):
                par = v & 1
                kb = kbO if par else kbE
                vp = vpO if par else vpE
                off = v - par  # even

                first = v == 0
                last = v == K - 1
                mm_args = []
                e_half = {}
                for u0, nu in HALVES:
                    s = sp.tile([128, nu, H, W], f16, tag=f"s{u0}")
                    nc.vector.tensor_mul(
                        s[:], q_bcast(nu), window_ap(kb, off, u0, nu, PW + RS))
                    e = ep.tile([128, nu, H, W], bf16, tag=f"e{u0}")
                    nc.scalar.activation(
                        e[:], s[:], mybir.ActivationFunctionType.Exp)
                    e_half[u0] = e
                    if last:
                        # tail: den accumulation straight after exp so
                        # it runs during the remaining m muls and the
                        # reciprocal unblocks early; sorted by chunk.
                        for want_ch in range(2):
                            csl = slice(want_ch * 512, (want_ch + 1) * 512)
                            for du in range(nu):
                                u = u0 + du
                                nc.tensor.matmul(
                                    dps[:, csl], isb[:],
                                    e[:, du, 16 * want_ch:16 * want_ch + 16, :],
                                    start=False, stop=(u == K - 1),
                                    skip_group_check=True,
                                )
                    m = mp.tile([128, nu, H, W], bf16, tag=f"m{u0}")
                    nc.vector.tensor_mul(
                        m[:], e[:], window_ap(vp, off, u0, nu, RS))
                    for du in range(nu):
                        u = u0 + du
                        for ch in range(2):
                            csl = slice(ch * 512, (ch + 1) * 512)
                            rsl = slice(16 * ch, 16 * ch + 16)
                            mm_args.append((csl, rsl, e, m, du, u))

                # den first so the tail's reciprocal (which needs only
                # dps) unblocks before the last num matmuls retire.
                if last:
                    # den already emitted above; num sorted by chunk.
                    for want_ch in (0, 1):
                        for csl, rsl, te, tm, du, u in mm_args:
                            if csl.start != want_ch * 512:
                                continue
                            nc.tensor.matmul(
                                nps[:, csl], isb[:], tm[:, du, rsl, :],
                                start=False, stop=(u == K - 1),
                                skip_group_check=True,
                            )
                else:
                    for csl, rsl, te, tm, du, u in mm_args:
                        for kind in ("den", "num"):
                            t = te if kind == "den" else tm
                            acc = dps if kind == "den" else nps
                            nc.tensor.matmul(
                                acc[:, csl], isb[:], t[:, du, rsl, :],
                                start=(first and u == 0),
                                stop=False,
                                skip_group_check=True,
                            )

            # ---- normalize and store: both reciprocals first (they
            # overlap the PE's final num matmuls), then multiply + DMA
            # per half on separate queues.
            rden = fin.tile([128, HW], f32)
            outsb = fin.tile([128, HW], f32)
            for ch in (0, 1):
                csl = slice(ch * 512, (ch + 1) * 512)
                nc.vector.reciprocal_approx_fast(
                    out=rden[:, csl], in_=dps[:, csl])
            for ch, eng in ((0, nc.sync), (1, nc.scalar)):
                csl = slice(ch * 512, (ch + 1) * 512)
                nc.vector.tensor_mul(
                    outsb[:, csl], nps[:, csl], rden[:, csl])
                eng.dma_start(out=o_ext[:, csl], in_=outsb[:, csl])

    nc.finalize()
    return nc


def _get_nc():
    if "nc" not in _NC_CACHE:
        _NC_CACHE["nc"] = _build_nc()
    return _NC_CACHE["nc"]


def _prep_in_maps(x, wq, wk, wv, rel_h, rel_w):
    bf = ml_dtypes.bfloat16
    ident = np.eye(128, dtype=bf)
    in_maps = []
    for core in range(N_CORES):
        b, cg = divmod(core, 2)
        xb = np.asarray(x[b], dtype=np.float32)
        if cg == 1:
            xb = xb.transpose(0, 2, 1)
        xb = np.ascontiguousarray(xb).reshape(2, 128, HW).astype(np.float16)
        rows = slice(cg * 128, (cg + 1) * 128)
        wt = np.stack([np.asarray(wq)[rows], np.asarray(wk)[rows],
                       np.asarray(wv)[rows]])          # [3, 128, 256]
        wt = np.ascontiguousarray(
            wt.transpose(0, 2, 1).astype(np.float16)).reshape(
            3, 2, 128, 128)                            # [wi, ci_chunk, ci, co]
        bias = np.ascontiguousarray(
            np.asarray(rel_h if cg == 0 else rel_w, dtype=np.float32))
        in_maps.append({"x": xb, "w": wt, "bias": bias, "ident": ident})
    return in_maps


def _assemble(results):
    out = np.empty((B, C, H, W), np.float32)
    for core in range(N_CORES):
        b, cg = divmod(core, 2)
        o = results[core]["out"].reshape(128, H, W)
        if cg == 1:
            o = o.transpose(0, 2, 1)
        out[b, cg * 128:(cg + 1) * 128] = o
    return out


def run(inputs, trace=False):
    """Returns (output, BassKernelResults)."""
    from concourse import bass_utils

    nc = _get_nc()
    in_maps = _prep_in_maps(**inputs)
    last_err = None
    for _attempt in range(3):
        try:
            res = bass_utils.run_bass_kernel_spmd(
                nc, in_maps, core_ids=list(range(N_CORES)), trace=trace)
            return _assemble(res.results), res
        except Exception as err:  # transient NRT device errors
            last_err = err
    raise last_err


def kernel(x, wq, wk, wv, rel_h, rel_w):
    out, _ = run(
        dict(x=x, wq=wq, wk=wk, wv=wv, rel_h=rel_h, rel_w=rel_w),
        trace=bool(os.environ.get("ATTNCONV_TRACE")),
    )
    return out


# revision 16
# speedup vs baseline: 1.1608x; 1.0029x over previous
"""AttentionConv (sparse local attention, 7x7 window, per-channel softmax)
Trainium2 Bass kernel, SPMD across 8 NeuronCores.

Sharding: core i handles batch b = i//2 and channel half cg = i%2
(channels are independent through the whole op: 1x1 convs produce each
output channel from all input channels, and the softmax is per-channel
over the 7x7 window).

The relative-position bias for channels [0,128) is rel_h[u] (window row)
and for channels [128,256) is rel_w[v] (window col). To keep one SPMD
program for all cores, cg=1 cores receive spatially TRANSPOSED x (H<->W)
and their output is transposed back on the host; under that transpose
rel_w becomes a window-row bias, identical in structure to cg=0.

Per-core pipeline (fp16 score path, bf16 value path, f32 accumulate):
  1. PE GEMMs: q,k,v = W @ x in fp16 (negligible rounding vs fp32 for
     this data, half the DMA bytes, 16-bit matmul speed). K=256
     contraction in 2 chunks, N chunks of 512 (one PSUM bank each).
  2. k,v scattered into zero-padded 38x40 planes; each plane stored
     twice (interior at col 3 and col 2) so windowed reads for even AND
     odd window-cols are 4-byte aligned -> DVE 16-bit 2x perf mode.
  3. 7 bias-added copies of each padded k plane (bias for a fixed
     window-row is a per-partition scalar -> tensor_scalar 4x mode),
     trimmed to the 32 rows each window-row actually reads.
  4. main loop over window col v (7 iters), u-dim split in (4,3) halves
     for pipelining, diagonal access patterns covering all u at once:
       s = q * k_biased[window]  fp16     (DVE TT, 2x)
       e = exp(s) -> bf16                 (ScalarE ACT, unnormalized --
                                           scores are far inside exp's
                                           f32/bf16 range, so no
                                           max-subtraction pass needed)
       m = e * v[window]  bf16            (DVE TT, 2x)
       num += I @ m ; den += I @ e        (TensorE identity matmuls
                                           accumulating in PSUM f32;
                                           the otherwise-idle PE does
                                           all the j-summation work)
  5. out = num * reciprocal_approx_fast(den); split-queue DMA out.
Engine budget per core: DVE ~62us (bottleneck: 2 multiplies per window
element at 2 elem/cyc/lane), ACT ~46us, PE ~52us, ~85us measured total.
NOTE (this session): aggressive software-pipelining of the DVE order
and STT-fused bias both REGRESS on HW (98.6us / 111us): overlapping
DVE+ACT+PE more densely inflates every op 8-25% (SBUF port contention),
and scalar_tensor_tensor has no 2x uop. Keep the baseline schedule;
only local, contention-neutral deltas win.
"""

import os

import numpy as np
import ml_dtypes

K = 7
PAD = 3
H = W = 32
HW = H * W
B = 4
C = 256
RS = 40          # padded plane row stride (elements); even => alignment
PR = H + 2 * PAD  # 38 padded rows
PW = PR * RS     # padded plane size per partition
N_CORES = 8

_NC_CACHE = {}


def _build_nc():
    import concourse.bass as bass
    import concourse.tile as tile
    from concourse import mybir, bacc

    bf16 = mybir.dt.bfloat16
    f16 = mybir.dt.float16
    f32 = mybir.dt.float32

    nc = bacc.Bacc(None)
    x_ext = nc.dram_tensor("x", [2, 128, HW], f16, kind="ExternalInput")
    w_ext = nc.dram_tensor("w", [3, 2, 128, 128], f16, kind="ExternalInput")
    b_ext = nc.dram_tensor("bias", [128, K], f32, kind="ExternalInput")
    i_ext = nc.dram_tensor("ident", [128, 128], bf16, kind="ExternalInput")
    o_ext = nc.dram_tensor("out", [128, HW], f32, kind="ExternalOutput")

    with tile.TileContext(nc) as tc:
        with (
            tc.tile_pool(name="consts", bufs=1) as consts,
            tc.tile_pool(name="kv", bufs=1) as kv,
            tc.tile_pool(name="fin", bufs=1) as fin,
            tc.tile_pool(name="psa", bufs=1, space="PSUM") as psa,
            tc.tile_pool(name="gt", bufs=1) as gt,
            tc.tile_pool(name="psg", bufs=4, space="PSUM") as psg,
            tc.tile_pool(name="sp", bufs=3) as sp,
            tc.tile_pool(name="ep", bufs=3) as ep,
            tc.tile_pool(name="mp", bufs=3) as mp,
        ):
            # DMAs spread across engine queues so they don't serialize.
            xsb = gt.tile([128, 2, HW], f16)
            wsb = gt.tile([128, 3, 2, 128], f16)
            bsb = consts.tile([128, K], f32)
            isb = consts.tile([128, 128], bf16)
            # DMA plan: per-queue DMA bandwidth is only ~85 GB/s, so the
            # 512KB of x is the startup gate -- spread it over all THREE
            # queues (x tails ride gpsimd behind ident). k-GEMM inputs
            # (w_k + x firsts) take the first two slots on the fast
            # queues; ident stays first overall (gates the PE warm-ups,
            # which also start the PE frequency ramp).
            nc.gpsimd.dma_start(out=isb[:], in_=i_ext[:])
            nc.sync.dma_start(out=wsb[:, 1, 0, :], in_=w_ext[1, 0])
            nc.scalar.dma_start(out=wsb[:, 1, 1, :], in_=w_ext[1, 1])
            nc.sync.dma_start(out=xsb[:, 0, 0:512], in_=x_ext[0][:, 0:512])
            nc.scalar.dma_start(out=xsb[:, 1, 0:512], in_=x_ext[1][:, 0:512])
            nc.gpsimd.dma_start(out=xsb[:, 0, 512:HW], in_=x_ext[0][:, 512:HW])
            nc.gpsimd.dma_start(out=xsb[:, 1, 512:HW], in_=x_ext[1][:, 512:HW])
            nc.gpsimd.dma_start(out=bsb[:], in_=b_ext[:])
            nc.sync.dma_start(out=wsb[:, 0, 0, :], in_=w_ext[0, 0])
            nc.scalar.dma_start(out=wsb[:, 0, 1, :], in_=w_ext[0, 1])
            nc.sync.dma_start(out=wsb[:, 2, 0, :], in_=w_ext[2, 0])
            nc.scalar.dma_start(out=wsb[:, 2, 1, :], in_=w_ext[2, 1])

            # q plane (read via a 7-way broadcast AP in the main loop)
            qsb = kv.tile([128, H, W], f16)
            # padded k/v planes; E holds interior at col 3 (for even v
            # window reads), O at col 2 (odd v reads at offset v-1).
            kpE = gt.tile([128, PR, RS], f16)
            kpO = gt.tile([128, PR, RS], f16)
            vpE = kv.tile([128, PR, RS], bf16)
            vpO = kv.tile([128, PR, RS], bf16)
            kbE = kv.tile([128, K, PR, RS], f16)
            kbO = kv.tile([128, K, PR, RS], f16)

            # k-plane zero-init: only the PAD region needs zeros (the
            # interior is overwritten by the GEMM scatter), and the
            # otherwise-idle-at-startup DVE does it off the gpsimd/scalar
            # queues' critical paths. v-planes on the scalar queue after
            # its DMAs.
            for kp, c0 in ((kpE, 3), (kpO, 2)):  # interior at cols c0..c0+31
                nc.vector.memset(kp[:, 0:PAD, :], 0.0)           # top rows
                nc.vector.memset(kp[:, PAD + H:PR, :], 0.0)      # bottom rows
                nc.vector.memset(kp[:, PAD:PAD + H, 0:c0], 0.0)  # left cols
                nc.vector.memset(kp[:, PAD:PAD + H, c0 + H:RS], 0.0)  # right
            nc.scalar.memzero(vpE[:])
            nc.scalar.memzero(vpO[:])

            nps = psa.tile([128, HW], f32)
            dps = psa.tile([128, HW], f32)

            # PE pipeline/HAM warm-up: dummy matmuls into nps, whose
            # content is discarded when the first start=True accumulation
            # clears has_written.
            for _ in range(3):
                nc.tensor.matmul(nps[:, 0:128], isb[:], isb[:],
                                 start=True, stop=True, skip_group_check=True)

            # ---- GEMMs: wi 0=q, 1=k, 2=v; N chunks of 512 px (16 rows)
            # k first: it gates the longest pre-loop chain (bias copies).
            for wi in (1, 0, 2):
                for ch in range(2):
                    ps = psg.tile([128, 16, 32], f32)
                    for ci in range(2):
                        nc.tensor.matmul(
                            ps[:],
                            wsb[:, wi, ci, :],
                            xsb[:, ci, ch * 512:(ch + 1) * 512],
                            start=(ci == 0),
                            stop=(ci == 1),
                        )
                    r0 = PAD + 16 * ch
                    if wi == 0:
                        nc.scalar.copy(qsb[:, 16 * ch:16 * ch + 16, :], ps[:])
                    elif wi == 1:
                        nc.vector.tensor_copy(kpE[:, r0:r0 + 16, 3:35], ps[:])
                        nc.scalar.copy(kpO[:, r0:r0 + 16, 2:34], ps[:])
                    else:
                        nc.scalar.copy(vpE[:, r0:r0 + 16, 3:35], ps[:])
                        nc.scalar.copy(vpO[:, r0:r0 + 16, 2:34], ps[:])

            # biased k copies: kb*[u] = kp* + bias[:, u] (per-partition),
            # trimmed to rows u..u+31 -- the only rows window-row u reads.
            # E copies here; the O copies (needed only from v=1) are
            # emitted inside the v=0 prologue, split around the first m
            # mul, so the first scores+exp start ~3us earlier.
            for u in range(K):
                nc.vector.tensor_scalar_add(
                    kbE[:, u, u:u + H, :], kpE[:, u:u + H, :], bsb[:, u:u + 1])

            def bias_copy_O(u):
                nc.vector.tensor_scalar_add(
                    kbO[:, u, u:u + H, :], kpO[:, u:u + H, :], bsb[:, u:u + 1])

            def window_ap(t, base_off, u0, nu, u_step):
                full = t[:]
                return bass.AP(
                    tensor=full.tensor,
                    offset=full.offset + base_off + u0 * u_step,
                    ap=[full.ap[0], [u_step, nu], [RS, H], [1, W]],
                )

            def q_bcast(nu):
                full = qsb[:]
                return bass.AP(
                    tensor=full.tensor,
                    offset=full.offset,
                    ap=[full.ap[0], [0, nu], [W, H], [1, W]],
                )

            # u-dim halves for finer DVE->ACT->PE pipelining. Measured
            # optimum: full-width ops 89.8us, quarters 87.2us, (3,4)
            # order 88.2us, this (4,3) split 85.4us. GpSimd offloads of
            # any slab regress (its in-loop TT is 3-5x slower than DVE).
            HALVES = ((0, 4), (4, 3))

            # ---- main loop over window col v
            def emit_se(v, u0, nu):
                kb = kbO if v & 1 else kbE
                off = v - (v & 1)
                s = sp.tile([128, nu, H, W], f16, tag=f"s{u0}")
                nc.vector.tensor_mul(
                    s[:], q_bcast(nu), window_ap(kb, off, u0, nu, PW + RS))
                e = ep.tile([128, nu, H, W], bf16, tag=f"e{u0}")
                nc.scalar.activation(
                    e[:], s[:], mybir.ActivationFunctionType.Exp)
                return e

            def emit_m_mm(v, u0, nu, e, mm_args):
                vp = vpO if v & 1 else vpE
                off = v - (v & 1)
                first = v == 0
                m = mp.tile([128, nu, H, W], bf16, tag=f"m{u0}")
                nc.vector.tensor_mul(
                    m[:], e[:], window_ap(vp, off, u0, nu, RS))
                for du in range(nu):
                    u = u0 + du
                    for ch in range(2):
                        csl = slice(ch * 512, (ch + 1) * 512)
                        rsl = slice(16 * ch, 16 * ch + 16)
                        mm_args.append((csl, rsl, e, m, du, u))
                if v < K - 1:
                    for csl, rsl, te, tm, du, u in mm_args[-2 * nu:]:
                        for kind in ("den", "num"):
                            t = te if kind == "den" else tm
                            acc = dps if kind == "den" else nps
                            nc.tensor.matmul(
                                acc[:, csl], isb[:], t[:, du, rsl, :],
                                start=(first and u == 0),
                                stop=False,
                                skip_group_check=True,
                            )

            # v=0 prologue: scores+exp for both halves go first; the kbO
            # copies (not needed until v=1) fill the DVE while the exp
            # latency drains, split around the first m mul.
            mm_args0 = []
            e00 = emit_se(0, *HALVES[0])
            e01 = emit_se(0, *HALVES[1])
            for u in range(4):
                bias_copy_O(u)
            emit_m_mm(0, HALVES[0][0], HALVES[0][1], e00, mm_args0)
            for u in range(4, K):
                bias_copy_O(u)
            emit_m_mm(0, HALVES[1][0], HALVES[1][1], e01, mm_args0)

            for v in range(1, K):
                last = v == K - 1
                mm_args = []
                for u0, nu in HALVES:
                    e = emit_se(v, u0, nu)
                    if last:
                        # tail: den accumulation straight after exp so
                        # it runs during the remaining m muls and the
                        # reciprocal unblocks early; sorted by chunk.
                        for want_ch in range(2):
                            csl = slice(want_ch * 512, (want_ch + 1) * 512)
                            for du in range(nu):
                                u = u0 + du
                                nc.tensor.matmul(
                                    dps[:, csl], isb[:],
                                    e[:, du, 16 * want_ch:16 * want_ch + 16, :],
                                    start=False, stop=(u == K - 1),
                                    skip_group_check=True,
                                )
                    emit_m_mm(v, u0, nu, e, mm_args)

                if last:
                    # den already emitted above; num sorted by chunk.
                    for want_ch in (0, 1):
                        for csl, rsl, te, tm, du, u in mm_args:
                            if csl.start != want_ch * 512:
                                continue
                            nc.tensor.matmul(
                                nps[:, csl], isb[:], tm[:, du, rsl, :],
                                start=False, stop=(u == K - 1),
                                skip_group_check=True,
                            )

            # ---- normalize and store: both reciprocals first (they
            # overlap the PE's final num matmuls), then multiply + DMA
            # per half on separate queues.
            rden = fin.tile([128, HW], f32)
            outsb = fin.tile([128, HW], f32)
            for ch in (0, 1):
                csl = slice(ch * 512, (ch + 1) * 512)
                nc.vector.reciprocal_approx_fast(
                    out=rden[:, csl], in_=dps[:, csl])
            for ch, eng in ((0, nc.sync), (1, nc.scalar)):
                csl = slice(ch * 512, (ch + 1) * 512)
                nc.vector.tensor_mul(
                    outsb[:, csl], nps[:, csl], rden[:, csl])
                eng.dma_start(out=o_ext[:, csl], in_=outsb[:, csl])

    nc.finalize()
    return nc


def _get_nc():
    if "nc" not in _NC_CACHE:
        _NC_CACHE["nc"] = _build_nc()
    return _NC_CACHE["nc"]


def _prep_in_maps(x, wq, wk, wv, rel_h, rel_w):
    bf = ml_dtypes.bfloat16
    ident = np.eye(128, dtype=bf)
    in_maps = []
    for core in range(N_CORES):
        b, cg = divmod(core, 2)
        xb = np.asarray(x[b], dtype=np.float32)
        if cg == 1:
            xb = xb.transpose(0, 2, 1)
        xb = np.ascontiguousarray(xb).reshape(2, 128, HW).astype(np.float16)
        rows = slice(cg * 128, (cg + 1) * 128)
        wt = np.stack([np.asarray(wq)[rows], np.asarray(wk)[rows],
                       np.asarray(wv)[rows]])          # [3, 128, 256]
        wt = np.ascontiguousarray(
            wt.transpose(0, 2, 1).astype(np.float16)).reshape(
            3, 2, 128, 128)                            # [wi, ci_chunk, ci, co]
        bias = np.ascontiguousarray(
            np.asarray(rel_h if cg == 0 else rel_w, dtype=np.float32))
        in_maps.append({"x": xb, "w": wt, "bias": bias, "ident": ident})
    return in_maps


def _assemble(results):
    out = np.empty((B, C, H, W), np.float32)
    for core in range(N_CORES):
        b, cg = divmod(core, 2)
        o = results[core]["out"].reshape(128, H, W)
        if cg == 1:
            o = o.transpose(0, 2, 1)
        out[b, cg * 128:(cg + 1) * 128] = o
    return out


def run(inputs, trace=False):
    """Returns (output, BassKernelResults)."""
    from concourse import bass_utils

    nc = _get_nc()
    in_maps = _prep_in_maps(**inputs)
    last_err = None
    for _attempt in range(3):
        try:
            res = bass_utils.run_bass_kernel_spmd(
                nc, in_maps, core_ids=list(range(N_CORES)), trace=trace)
            return _assemble(res.results), res
        except Exception as err:  # transient NRT device errors
            last_err = err
    raise last_err


def kernel(x, wq, wk, wv, rel_h, rel_w):
    out, _ = run(
        dict(x=x, wq=wq, wk=wk, wv=wv, rel_h=rel_h, rel_w=rel_w),
        trace=bool(os.environ.get("ATTNCONV_TRACE")),
    )
    return out


# revision 18
# speedup vs baseline: 1.1751x; 1.0123x over previous
"""AttentionConv (sparse local attention, 7x7 window, per-channel softmax)
Trainium2 Bass kernel, SPMD across 8 NeuronCores.

Sharding: core i handles batch b = i//2 and channel half cg = i%2
(channels are independent through the whole op: 1x1 convs produce each
output channel from all input channels, and the softmax is per-channel
over the 7x7 window).

The relative-position bias for channels [0,128) is rel_h[u] (window row)
and for channels [128,256) is rel_w[v] (window col). To keep one SPMD
program for all cores, cg=1 cores receive spatially TRANSPOSED x (H<->W)
and their output is transposed back on the host; under that transpose
rel_w becomes a window-row bias, identical in structure to cg=0.

Per-core pipeline (fp16 score path, bf16 value path, f32 accumulate):
  1. PE GEMMs: q,k,v = W @ x in fp16 (negligible rounding vs fp32 for
     this data, half the DMA bytes, 16-bit matmul speed). K=256
     contraction in 2 chunks, N chunks of 512 (one PSUM bank each).
  2. k,v scattered into zero-padded 38x40 planes; each plane stored
     twice (interior at col 3 and col 2) so windowed reads for even AND
     odd window-cols are 4-byte aligned -> DVE 16-bit 2x perf mode.
  3. 7 bias-added copies of each padded k plane (bias for a fixed
     window-row is a per-partition scalar -> tensor_scalar 4x mode),
     trimmed to the 32 rows each window-row actually reads.
  4. main loop over window col v (7 iters), u-dim split in (4,3) halves
     for pipelining, diagonal access patterns covering all u at once:
       s = q * k_biased[window]  fp16     (DVE TT, 2x)
       e = exp(s) -> bf16                 (ScalarE ACT, unnormalized --
                                           scores are far inside exp's
                                           f32/bf16 range, so no
                                           max-subtraction pass needed)
       m = e * v[window]  bf16            (DVE TT, 2x)
       num += I @ m ; den += I @ e        (TensorE identity matmuls
                                           accumulating in PSUM f32;
                                           the otherwise-idle PE does
                                           all the j-summation work)
  5. tail: den matmuls of the last v emitted right after its exp (so
     the reciprocal unblocks while the num matmuls retire), then
     out = num * reciprocal_approx_fast(den); split-queue DMA out.
Engine budget per core: DVE ~62us (bottleneck: 2 multiplies per window
element at 2 elem/cyc/lane), ACT ~46us, PE ~52us union-busy ~50us
(identity matmuls pipeline back-to-back at 216ns effective for 512
cols; trace "durations" overlap by ~163ns).

Measured (same-process paired A/B, fast device state): this version
84.3/84.8/84.3 vs the previous 85.3/85.8/86.7; best observed 83.6us.
Wins over the prior baseline: trimmed bias copies (1280 vs 1520 elems),
v-plane memzero moved to the scalar queue (unserializes gpsimd ahead of
the k scatter), last-v den-before-m tail.

Negative results (all verified on HW, some with paired A/B):
 - scalar_tensor_tensor (fused bias+mul) has no 2x uop -> 1x, s-path
   doubles (111us).
 - Denser schedules REGRESS ~20% per-op across ALL engines (power/DVFS
   throttling, not dependency stalls): full one-v-ahead software
   pipelining 98.7us reproducible; kbO-copy split around the first m
   (starts the loop 1.7us earlier) is span-neutral.
 - PE warm-up fill (24 dummy matmuls to pre-ramp the PE clock) delays
   the cold GEMMs instead: +2us.
 - Merged den+num matmuls via [2,512] PSUM out AP: NCC ISA check
   rejects multi-bank matmul writes.
 - x-DMA spread over 3 queues: neutral-to-negative (per-queue DMA is
   ~85-125GB/s; k-path already first in line).
Beware: the device toggles between a fast (~84us) and a slow (~100us)
state that persists for a whole process; only same-process paired
comparisons are trustworthy (in-process rep spread is ~300ns).
"""

import os

import numpy as np
import ml_dtypes

K = 7
PAD = 3
H = W = 32
HW = H * W
B = 4
C = 256
RS = 40          # padded plane row stride (elements); even => alignment
PR = H + 2 * PAD  # 38 padded rows
PW = PR * RS     # padded plane size per partition
N_CORES = 8

_NC_CACHE = {}


def _build_nc():
    import concourse.bass as bass
    import concourse.tile as tile
    from concourse import mybir, bacc

    bf16 = mybir.dt.bfloat16
    f16 = mybir.dt.float16
    f32 = mybir.dt.float32

    nc = bacc.Bacc(None)
    x_ext = nc.dram_tensor("x", [2, 128, HW], f16, kind="ExternalInput")
    w_ext = nc.dram_tensor("w", [3, 2, 128, 128], f16, kind="ExternalInput")
    b_ext = nc.dram_tensor("bias", [128, K], f32, kind="ExternalInput")
    i_ext = nc.dram_tensor("ident", [128, 128], bf16, kind="ExternalInput")
    o_ext = nc.dram_tensor("out", [128, HW], f32, kind="ExternalOutput")

    with tile.TileContext(nc) as tc:
        with (
            tc.tile_pool(name="consts", bufs=1) as consts,
            tc.tile_pool(name="kv", bufs=1) as kv,
            tc.tile_pool(name="fin", bufs=1) as fin,
            tc.tile_pool(name="psa", bufs=1, space="PSUM") as psa,
            tc.tile_pool(name="gt", bufs=1) as gt,
            tc.tile_pool(name="psg", bufs=4, space="PSUM") as psg,
            tc.tile_pool(name="sp", bufs=3) as sp,
            tc.tile_pool(name="ep", bufs=3) as ep,
            tc.tile_pool(name="mp", bufs=3) as mp,
        ):
            # DMAs spread across engine queues so they don't serialize.
            xsb = gt.tile([128, 2, HW], f16)
            wsb = gt.tile([128, 3, 2, 128], f16)
            bsb = consts.tile([128, K], f32)
            isb = consts.tile([128, 128], bf16)
            # k-GEMM path first on both fast queues; x split into halves
            # so the first k matmuls' data lands early.
            nc.gpsimd.dma_start(out=isb[:], in_=i_ext[:])
            nc.sync.dma_start(out=wsb[:, 1, 0, :], in_=w_ext[1, 0])
            nc.scalar.dma_start(out=wsb[:, 1, 1, :], in_=w_ext[1, 1])
            # (q and v weights each as one DMA below)
            nc.sync.dma_start(out=xsb[:, 0, 0:512], in_=x_ext[0][:, 0:512])
            nc.scalar.dma_start(out=xsb[:, 1, 0:512], in_=x_ext[1][:, 0:512])
            nc.sync.dma_start(out=xsb[:, 0, 512:HW], in_=x_ext[0][:, 512:HW])
            nc.scalar.dma_start(out=xsb[:, 1, 512:HW], in_=x_ext[1][:, 512:HW])
            nc.sync.dma_start(out=bsb[:], in_=b_ext[:])
            nc.scalar.dma_start(out=wsb[:, 0, 1, :], in_=w_ext[0, 1])
            nc.sync.dma_start(out=wsb[:, 0, 0, :], in_=w_ext[0, 0])
            nc.scalar.dma_start(out=wsb[:, 2, 1, :], in_=w_ext[2, 1])
            nc.sync.dma_start(out=wsb[:, 2, 0, :], in_=w_ext[2, 0])

            # q plane (read via a 7-way broadcast AP in the main loop)
            qsb = kv.tile([128, H, W], f16)
            # padded k/v planes; E holds interior at col 3 (for even v
            # window reads), O at col 2 (odd v reads at offset v-1).
            kpE = gt.tile([128, PR, RS], f16)
            kpO = gt.tile([128, PR, RS], f16)
            vpE = kv.tile([128, PR, RS], bf16)
            vpO = kv.tile([128, PR, RS], bf16)
            kbE = kv.tile([128, K, PR, RS], f16)
            kbO = kv.tile([128, K, PR, RS], f16)

            # k-plane zero-init on gpsimd; v-planes on the scalar queue
            # after its DMAs (frees ~2.8us of serial gpsimd memset time
            # ahead of the k scatter).
            nc.gpsimd.memset(kpE[:], 0.0)
            nc.gpsimd.memset(kpO[:], 0.0)
            nc.scalar.memzero(vpE[:])
            nc.scalar.memzero(vpO[:])

            nps = psa.tile([128, HW], f32)
            dps = psa.tile([128, HW], f32)

            # PE pipeline/HAM warm-up: dummy matmuls into nps, whose
            # content is discarded when the first start=True accumulation
            # clears has_written.
            for _ in range(3):
                nc.tensor.matmul(nps[:, 0:128], isb[:], isb[:],
                                 start=True, stop=True, skip_group_check=True)

            # ---- GEMMs: wi 0=q, 1=k, 2=v; N chunks of 512 px (16 rows)
            # k first: it gates the longest pre-loop chain (bias copies).
            for wi in (1, 0, 2):
                for ch in range(2):
                    ps = psg.tile([128, 16, 32], f32)
                    for ci in range(2):
                        nc.tensor.matmul(
                            ps[:],
                            wsb[:, wi, ci, :],
                            xsb[:, ci, ch * 512:(ch + 1) * 512],
                            start=(ci == 0),
                            stop=(ci == 1),
                        )
                    r0 = PAD + 16 * ch
                    if wi == 0:
                        nc.scalar.copy(qsb[:, 16 * ch:16 * ch + 16, :], ps[:])
                    elif wi == 1:
                        nc.vector.tensor_copy(kpE[:, r0:r0 + 16, 3:35], ps[:])
                        nc.scalar.copy(kpO[:, r0:r0 + 16, 2:34], ps[:])
                    else:
                        nc.scalar.copy(vpE[:, r0:r0 + 16, 3:35], ps[:])
                        nc.scalar.copy(vpO[:, r0:r0 + 16, 2:34], ps[:])

            # biased k copies: kb*[u] = kp* + bias[:, u] (per-partition),
            # trimmed to rows u..u+31 -- the only rows window-row u reads.
            for u in range(K):
                nc.vector.tensor_scalar_add(
                    kbE[:, u, u:u + H, :], kpE[:, u:u + H, :], bsb[:, u:u + 1])
            for u in range(K):
                nc.vector.tensor_scalar_add(
                    kbO[:, u, u:u + H, :], kpO[:, u:u + H, :], bsb[:, u:u + 1])

            def window_ap(t, base_off, u0, nu, u_step):
                full = t[:]
                return bass.AP(
                    tensor=full.tensor,
                    offset=full.offset + base_off + u0 * u_step,
                    ap=[full.ap[0], [u_step, nu], [RS, H], [1, W]],
                )

            def q_bcast(nu):
                full = qsb[:]
                return bass.AP(
                    tensor=full.tensor,
                    offset=full.offset,
                    ap=[full.ap[0], [0, nu], [W, H], [1, W]],
                )

            # u-dim halves for finer DVE->ACT->PE pipelining. Measured
            # optimum: full-width ops 89.8us, quarters 87.2us, (3,4)
            # order 88.2us, this (4,3) split 85.4us. GpSimd offloads of
            # any slab regress (its in-loop TT is 3-5x slower than DVE).
            HALVES = ((0, 4), (4, 3))

            # ---- main loop over window col v
            for v in range(K):
                par = v & 1
                kb = kbO if par else kbE
                vp = vpO if par else vpE
                off = v - par  # even

                first = v == 0
                last = v == K - 1
                mm_args = []
                e_half = {}
                for u0, nu in HALVES:
                    s = sp.tile([128, nu, H, W], f16, tag=f"s{u0}")
                    nc.vector.tensor_mul(
                        s[:], q_bcast(nu), window_ap(kb, off, u0, nu, PW + RS))
                    e = ep.tile([128, nu, H, W], bf16, tag=f"e{u0}")
                    nc.scalar.activation(
                        e[:], s[:], mybir.ActivationFunctionType.Exp)
                    e_half[u0] = e
                    if last:
                        # tail: den accumulation straight after exp so
                        # it runs during the remaining m muls and the
                        # reciprocal unblocks early; sorted by chunk.
                        for want_ch in range(2):
                            csl = slice(want_ch * 512, (want_ch + 1) * 512)
                            for du in range(nu):
                                u = u0 + du
                                nc.tensor.matmul(
                                    dps[:, csl], isb[:],
                                    e[:, du, 16 * want_ch:16 * want_ch + 16, :],
                                    start=False, stop=(u == K - 1),
                                    skip_group_check=True,
                                )
                    m = mp.tile([128, nu, H, W], bf16, tag=f"m{u0}")
                    nc.vector.tensor_mul(
                        m[:], e[:], window_ap(vp, off, u0, nu, RS))
                    for du in range(nu):
                        u = u0 + du
                        for ch in range(2):
                            csl = slice(ch * 512, (ch + 1) * 512)
                            rsl = slice(16 * ch, 16 * ch + 16)
                            mm_args.append((csl, rsl, e, m, du, u))

                # den first so the tail's reciprocal (which needs only
                # dps) unblocks before the last num matmuls retire.
                if last:
                    # den already emitted above; num sorted by chunk.
                    for want_ch in (0, 1):
                        for csl, rsl, te, tm, du, u in mm_args:
                            if csl.start != want_ch * 512:
                                continue
                            nc.tensor.matmul(
                                nps[:, csl], isb[:], tm[:, du, rsl, :],
                                start=False, stop=(u == K - 1),
                                skip_group_check=True,
                            )
                else:
                    for csl, rsl, te, tm, du, u in mm_args:
                        for kind in ("den", "num"):
                            t = te if kind == "den" else tm
                            acc = dps if kind == "den" else nps
                            nc.tensor.matmul(
                                acc[:, csl], isb[:], t[:, du, rsl, :],
                                start=(first and u == 0),
                                stop=False,
                                skip_group_check=True,
                            )

            # ---- normalize and store: both reciprocals first (they
            # overlap the PE's final num matmuls), then multiply + DMA
            # per half on separate queues.
            rden = fin.tile([128, HW], f32)
            outsb = fin.tile([128, HW], f32)
            for ch in (0, 1):
                csl = slice(ch * 512, (ch + 1) * 512)
                nc.vector.reciprocal_approx_fast(
                    out=rden[:, csl], in_=dps[:, csl])
            for ch, eng in ((0, nc.sync), (1, nc.scalar)):
                csl = slice(ch * 512, (ch + 1) * 512)
                nc.vector.tensor_mul(
                    outsb[:, csl], nps[:, csl], rden[:, csl])
                eng.dma_start(out=o_ext[:, csl], in_=outsb[:, csl])

    nc.finalize()
    return nc


def _get_nc():
    if "nc" not in _NC_CACHE:
        _NC_CACHE["nc"] = _build_nc()
    return _NC_CACHE["nc"]


def _prep_in_maps(x, wq, wk, wv, rel_h, rel_w):
    bf = ml_dtypes.bfloat16
    ident = np.eye(128, dtype=bf)
    in_maps = []
    for core in range(N_CORES):
        b, cg = divmod(core, 2)
        xb = np.asarray(x[b], dtype=np.float32)
        if cg == 1:
            xb = xb.transpose(0, 2, 1)
        xb = np.ascontiguousarray(xb).reshape(2, 128, HW).astype(np.float16)
        rows = slice(cg * 128, (cg + 1) * 128)
        wt = np.stack([np.asarray(wq)[rows], np.asarray(wk)[rows],
                       np.asarray(wv)[rows]])          # [3, 128, 256]
        wt = np.ascontiguousarray(
            wt.transpose(0, 2, 1).astype(np.float16)).reshape(
            3, 2, 128, 128)                            # [wi, ci_chunk, ci, co]
        bias = np.ascontiguousarray(
            np.asarray(rel_h if cg == 0 else rel_w, dtype=np.float32))
        in_maps.append({"x": xb, "w": wt, "bias": bias, "ident": ident})
    return in_maps


def _assemble(results):
    out = np.empty((B, C, H, W), np.float32)
    for core in range(N_CORES):
        b, cg = divmod(core, 2)
        o = results[core]["out"].reshape(128, H, W)
        if cg == 1:
            o = o.transpose(0, 2, 1)
        out[b, cg * 128:(cg + 1) * 128] = o
    return out


def run(inputs, trace=False):
    """Returns (output, BassKernelResults)."""
    from concourse import bass_utils

    nc = _get_nc()
    in_maps = _prep_in_maps(**inputs)
    last_err = None
    for _attempt in range(3):
        try:
            res = bass_utils.run_bass_kernel_spmd(
                nc, in_maps, core_ids=list(range(N_CORES)), trace=trace)
            return _assemble(res.results), res
        except Exception as err:  # transient NRT device errors
            last_err = err
    raise last_err


def kernel(x, wq, wk, wv, rel_h, rel_w):
    out, _ = run(
        dict(x=x, wq=wq, wk=wk, wv=wv, rel_h=rel_h, rel_w=rel_w),
        trace=bool(os.environ.get("ATTNCONV_TRACE")),
    )
    return out


# revision 19
# speedup vs baseline: 1.1851x; 1.0085x over previous
"""AttentionConv (sparse local attention, 7x7 window, per-channel softmax)
Trainium2 Bass kernel, SPMD across 8 NeuronCores.

Sharding: core i handles batch b = i//2 and channel half cg = i%2
(channels are independent through the whole op: 1x1 convs produce each
output channel from all input channels, and the softmax is per-channel
over the 7x7 window).

The relative-position bias for channels [0,128) is rel_h[u] (window row)
and for channels [128,256) is rel_w[v] (window col). To keep one SPMD
program for all cores, cg=1 cores receive spatially TRANSPOSED x (H<->W)
and their output is transposed back on the host; under that transpose
rel_w becomes a window-row bias, identical in structure to cg=0.

Per-core pipeline (fp16 score path, bf16 value path, f32 accumulate):
  1. PE GEMMs: q,k,v = W @ x in fp16 (negligible rounding vs fp32 for
     this data, half the DMA bytes, 16-bit matmul speed). K=256
     contraction in 2 chunks, N chunks of 512 (one PSUM bank each).
  2. k,v scattered into zero-padded 38x40 planes; each plane stored
     twice (interior at col 3 and col 2) so windowed reads for even AND
     odd window-cols are 4-byte aligned -> DVE 16-bit 2x perf mode.
  3. 7 bias-added copies of each padded k plane (bias for a fixed
     window-row is a per-partition scalar -> tensor_scalar 4x mode),
     trimmed to the 32 rows each window-row actually reads.
  4. main loop over window col v (7 iters), u-dim split in (4,3) halves
     for pipelining, diagonal access patterns covering all u at once:
       s = q * k_biased[window]  fp16     (DVE TT, 2x)
       e = exp(s) -> bf16                 (ScalarE ACT, unnormalized --
                                           scores are far inside exp's
                                           f32/bf16 range, so no
                                           max-subtraction pass needed)
       m = e * v[window]  bf16            (DVE TT, 2x)
       num += I @ m ; den += I @ e        (TensorE identity matmuls
                                           accumulating in PSUM f32;
                                           the otherwise-idle PE does
                                           all the j-summation work)
  5. out = num * reciprocal_approx_fast(den); split-queue DMA out.
Engine budget per core: DVE ~62us (bottleneck: 2 multiplies per window
element at 2 elem/cyc/lane), ACT ~46us, PE ~52us, ~85us measured total.
Measured (same-process paired A/B, fast device state): 83.27us on both
reps, vs 83.8/84.4 without the per-chunk PSUM tiles and 85.3-86.7 for
the session-start baseline (85.4us). Wins: trimmed bias copies (1280 vs
1520 elems), v-plane memzero on the scalar queue (unserializes gpsimd
ahead of the k scatter), last-v den-before-m tail, and PER-CHUNK PSUM
accumulators (nps0/1, dps0/1) -- Tile deps are tile-granular, so with
single [128,1024] accumulators the ch0 reciprocal/outmul sat ~1.5us
waiting for ch1's matmuls (ew=1420 on the first outmul in the trace).

Negative results (verified on HW, mostly paired A/B):
 - scalar_tensor_tensor (fused bias+mul) has no 2x uop -> 1x (111us).
 - Denser schedules REGRESS ~20% per-op on ALL engines (power/DVFS
   throttling): full one-v-ahead DVE pipelining 98.7us reproducible;
   kbO-copy split around the first m is span-neutral (the Tile
   scheduler already pipelines across v boundaries).
 - PE warm-up fill delays the cold GEMMs (+2us); merged den+num
   matmuls ([2,512] PSUM out AP) fail the NCC ISA check; splitting the
   last out-DMA across two queues costs more than it saves (+0.15us);
   x-DMA over 3 queues neutral-to-negative (per-queue DMA ~85-125GB/s).
 - DMA cannot read PSUM (kills host-side-divide tail tricks).
Beware: the device toggles between a fast (~84us) and slow (~100us)
state per process; only same-process paired A/B is trustworthy
(in-process rep spread ~300ns, cross-process +-1.5us).
"""

import os

import numpy as np
import ml_dtypes

K = 7
PAD = 3
H = W = 32
HW = H * W
B = 4
C = 256
RS = 40          # padded plane row stride (elements); even => alignment
PR = H + 2 * PAD  # 38 padded rows
PW = PR * RS     # padded plane size per partition
N_CORES = 8

_NC_CACHE = {}


def _build_nc():
    import concourse.bass as bass
    import concourse.tile as tile
    from concourse import mybir, bacc

    bf16 = mybir.dt.bfloat16
    f16 = mybir.dt.float16
    f32 = mybir.dt.float32

    nc = bacc.Bacc(None)
    x_ext = nc.dram_tensor("x", [2, 128, HW], f16, kind="ExternalInput")
    w_ext = nc.dram_tensor("w", [3, 2, 128, 128], f16, kind="ExternalInput")
    b_ext = nc.dram_tensor("bias", [128, K], f32, kind="ExternalInput")
    i_ext = nc.dram_tensor("ident", [128, 128], bf16, kind="ExternalInput")
    o_ext = nc.dram_tensor("out", [128, HW], f32, kind="ExternalOutput")

    with tile.TileContext(nc) as tc:
        with (
            tc.tile_pool(name="consts", bufs=1) as consts,
            tc.tile_pool(name="kv", bufs=1) as kv,
            tc.tile_pool(name="fin", bufs=1) as fin,
            tc.tile_pool(name="psa", bufs=1, space="PSUM") as psa,
            tc.tile_pool(name="gt", bufs=1) as gt,
            tc.tile_pool(name="psg", bufs=4, space="PSUM") as psg,
            tc.tile_pool(name="sp", bufs=3) as sp,
            tc.tile_pool(name="ep", bufs=3) as ep,
            tc.tile_pool(name="mp", bufs=3) as mp,
        ):
            # DMAs spread across engine queues so they don't serialize.
            xsb = gt.tile([128, 2, HW], f16)
            wsb = gt.tile([128, 3, 2, 128], f16)
            bsb = consts.tile([128, K], f32)
            isb = consts.tile([128, 128], bf16)
            # k-GEMM path first on both fast queues; x split into halves
            # so the first k matmuls' data lands early.
            nc.gpsimd.dma_start(out=isb[:], in_=i_ext[:])
            nc.sync.dma_start(out=wsb[:, 1, 0, :], in_=w_ext[1, 0])
            nc.scalar.dma_start(out=wsb[:, 1, 1, :], in_=w_ext[1, 1])
            # (q and v weights each as one DMA below)
            nc.sync.dma_start(out=xsb[:, 0, 0:512], in_=x_ext[0][:, 0:512])
            nc.scalar.dma_start(out=xsb[:, 1, 0:512], in_=x_ext[1][:, 0:512])
            nc.sync.dma_start(out=xsb[:, 0, 512:HW], in_=x_ext[0][:, 512:HW])
            nc.scalar.dma_start(out=xsb[:, 1, 512:HW], in_=x_ext[1][:, 512:HW])
            nc.sync.dma_start(out=bsb[:], in_=b_ext[:])
            nc.scalar.dma_start(out=wsb[:, 0, 1, :], in_=w_ext[0, 1])
            nc.sync.dma_start(out=wsb[:, 0, 0, :], in_=w_ext[0, 0])
            nc.scalar.dma_start(out=wsb[:, 2, 1, :], in_=w_ext[2, 1])
            nc.sync.dma_start(out=wsb[:, 2, 0, :], in_=w_ext[2, 0])

            # q plane (read via a 7-way broadcast AP in the main loop)
            qsb = kv.tile([128, H, W], f16)
            # padded k/v planes; E holds interior at col 3 (for even v
            # window reads), O at col 2 (odd v reads at offset v-1).
            kpE = gt.tile([128, PR, RS], f16)
            kpO = gt.tile([128, PR, RS], f16)
            vpE = kv.tile([128, PR, RS], bf16)
            vpO = kv.tile([128, PR, RS], bf16)
            kbE = kv.tile([128, K, PR, RS], f16)
            kbO = kv.tile([128, K, PR, RS], f16)

            # k-plane zero-init on gpsimd; v-planes on the scalar queue
            # after its DMAs (frees ~2.8us of serial gpsimd memset time
            # ahead of the k scatter).
            nc.gpsimd.memset(kpE[:], 0.0)
            nc.gpsimd.memset(kpO[:], 0.0)
            nc.scalar.memzero(vpE[:])
            nc.scalar.memzero(vpO[:])

            # per-chunk PSUM accumulators: tile-granular deps mean the
            # ch0 reciprocal/outmul/DMA no longer wait for ch1's matmuls
            nps_t = [psa.tile([128, 512], f32, name=f"nps{c}") for c in range(2)]
            dps_t = [psa.tile([128, 512], f32, name=f"dps{c}") for c in range(2)]

            # PE pipeline/HAM warm-up: dummy matmuls into nps, whose
            # content is discarded when the first start=True accumulation
            # clears has_written.
            for _ in range(3):
                nc.tensor.matmul(nps_t[0][:, 0:128], isb[:], isb[:],
                                 start=True, stop=True, skip_group_check=True)

            # ---- GEMMs: wi 0=q, 1=k, 2=v; N chunks of 512 px (16 rows)
            # k first: it gates the longest pre-loop chain (bias copies).
            for wi in (1, 0, 2):
                for ch in range(2):
                    ps = psg.tile([128, 16, 32], f32)
                    for ci in range(2):
                        nc.tensor.matmul(
                            ps[:],
                            wsb[:, wi, ci, :],
                            xsb[:, ci, ch * 512:(ch + 1) * 512],
                            start=(ci == 0),
                            stop=(ci == 1),
                        )
                    r0 = PAD + 16 * ch
                    if wi == 0:
                        nc.scalar.copy(qsb[:, 16 * ch:16 * ch + 16, :], ps[:])
                    elif wi == 1:
                        nc.vector.tensor_copy(kpE[:, r0:r0 + 16, 3:35], ps[:])
                        nc.scalar.copy(kpO[:, r0:r0 + 16, 2:34], ps[:])
                    else:
                        nc.scalar.copy(vpE[:, r0:r0 + 16, 3:35], ps[:])
                        nc.scalar.copy(vpO[:, r0:r0 + 16, 2:34], ps[:])

            # biased k copies: kb*[u] = kp* + bias[:, u] (per-partition),
            # trimmed to rows u..u+31 -- the only rows window-row u reads.
            for u in range(K):
                nc.vector.tensor_scalar_add(
                    kbE[:, u, u:u + H, :], kpE[:, u:u + H, :], bsb[:, u:u + 1])
            for u in range(K):
                nc.vector.tensor_scalar_add(
                    kbO[:, u, u:u + H, :], kpO[:, u:u + H, :], bsb[:, u:u + 1])

            def window_ap(t, base_off, u0, nu, u_step):
                full = t[:]
                return bass.AP(
                    tensor=full.tensor,
                    offset=full.offset + base_off + u0 * u_step,
                    ap=[full.ap[0], [u_step, nu], [RS, H], [1, W]],
                )

            def q_bcast(nu):
                full = qsb[:]
                return bass.AP(
                    tensor=full.tensor,
                    offset=full.offset,
                    ap=[full.ap[0], [0, nu], [W, H], [1, W]],
                )

            # u-dim halves for finer DVE->ACT->PE pipelining. Measured
            # optimum: full-width ops 89.8us, quarters 87.2us, (3,4)
            # order 88.2us, this (4,3) split 85.4us. GpSimd offloads of
            # any slab regress (its in-loop TT is 3-5x slower than DVE).
            HALVES = ((0, 4), (4, 3))

            # ---- main loop over window col v
            for v in range(K):
                par = v & 1
                kb = kbO if par else kbE
                vp = vpO if par else vpE
                off = v - par  # even

                first = v == 0
                last = v == K - 1
                mm_args = []
                e_half = {}
                for u0, nu in HALVES:
                    s = sp.tile([128, nu, H, W], f16, tag=f"s{u0}")
                    nc.vector.tensor_mul(
                        s[:], q_bcast(nu), window_ap(kb, off, u0, nu, PW + RS))
                    e = ep.tile([128, nu, H, W], bf16, tag=f"e{u0}")
                    nc.scalar.activation(
                        e[:], s[:], mybir.ActivationFunctionType.Exp)
                    e_half[u0] = e
                    if last:
                        # tail: den accumulation straight after exp so
                        # it runs during the remaining m muls and the
                        # reciprocal unblocks early; sorted by chunk.
                        for want_ch in range(2):
                            for du in range(nu):
                                u = u0 + du
                                nc.tensor.matmul(
                                    dps_t[want_ch][:], isb[:],
                                    e[:, du, 16 * want_ch:16 * want_ch + 16, :],
                                    start=False, stop=(u == K - 1),
                                    skip_group_check=True,
                                )
                    m = mp.tile([128, nu, H, W], bf16, tag=f"m{u0}")
                    nc.vector.tensor_mul(
                        m[:], e[:], window_ap(vp, off, u0, nu, RS))
                    for du in range(nu):
                        u = u0 + du
                        for ch in range(2):
                            csl = slice(ch * 512, (ch + 1) * 512)
                            rsl = slice(16 * ch, 16 * ch + 16)
                            mm_args.append((csl, rsl, e, m, du, u))

                # den first so the tail's reciprocal (which needs only
                # dps) unblocks before the last num matmuls retire.
                if last:
                    # den already emitted above; num sorted by chunk.
                    for want_ch in (0, 1):
                        for csl, rsl, te, tm, du, u in mm_args:
                            if csl.start != want_ch * 512:
                                continue
                            nc.tensor.matmul(
                                nps_t[want_ch][:], isb[:], tm[:, du, rsl, :],
                                start=False, stop=(u == K - 1),
                                skip_group_check=True,
                            )
                else:
                    for csl, rsl, te, tm, du, u in mm_args:
                        ch = csl.start // 512
                        for kind in ("den", "num"):
                            t = te if kind == "den" else tm
                            acc = dps_t[ch] if kind == "den" else nps_t[ch]
                            nc.tensor.matmul(
                                acc[:], isb[:], t[:, du, rsl, :],
                                start=(first and u == 0),
                                stop=False,
                                skip_group_check=True,
                            )

            # ---- normalize and store: both reciprocals first (they
            # overlap the PE's final num matmuls), then multiply + DMA
            # per half on separate queues.
            rden = fin.tile([128, HW], f32)
            outsb = fin.tile([128, HW], f32)
            for ch in (0, 1):
                csl = slice(ch * 512, (ch + 1) * 512)
                nc.vector.reciprocal_approx_fast(
                    out=rden[:, csl], in_=dps_t[ch][:])
            for ch, eng in ((0, nc.sync), (1, nc.scalar)):
                csl = slice(ch * 512, (ch + 1) * 512)
                nc.vector.tensor_mul(
                    outsb[:, csl], nps_t[ch][:], rden[:, csl])
                eng.dma_start(out=o_ext[:, csl], in_=outsb[:, csl])

    nc.finalize()
    return nc


def _get_nc():
    if "nc" not in _NC_CACHE:
        _NC_CACHE["nc"] = _build_nc()
    return _NC_CACHE["nc"]


def _prep_in_maps(x, wq, wk, wv, rel_h, rel_w):
    bf = ml_dtypes.bfloat16
    ident = np.eye(128, dtype=bf)
    in_maps = []
    for core in range(N_CORES):
        b, cg = divmod(core, 2)
        xb = np.asarray(x[b], dtype=np.float32)
        if cg == 1:
            xb = xb.transpose(0, 2, 1)
        xb = np.ascontiguousarray(xb).reshape(2, 128, HW).astype(np.float16)
        rows = slice(cg * 128, (cg + 1) * 128)
        wt = np.stack([np.asarray(wq)[rows], np.asarray(wk)[rows],
                       np.asarray(wv)[rows]])          # [3, 128, 256]
        wt = np.ascontiguousarray(
            wt.transpose(0, 2, 1).astype(np.float16)).reshape(
            3, 2, 128, 128)                            # [wi, ci_chunk, ci, co]
        bias = np.ascontiguousarray(
            np.asarray(rel_h if cg == 0 else rel_w, dtype=np.float32))
        in_maps.append({"x": xb, "w": wt, "bias": bias, "ident": ident})
    return in_maps


def _assemble(results):
    out = np.empty((B, C, H, W), np.float32)
    for core in range(N_CORES):
        b, cg = divmod(core, 2)
        o = results[core]["out"].reshape(128, H, W)
        if cg == 1:
            o = o.transpose(0, 2, 1)
        out[b, cg * 128:(cg + 1) * 128] = o
    return out


def run(inputs, trace=False):
    """Returns (output, BassKernelResults)."""
    from concourse import bass_utils

    nc = _get_nc()
    in_maps = _prep_in_maps(**inputs)
    last_err = None
    for _attempt in range(3):
        try:
            res = bass_utils.run_bass_kernel_spmd(
                nc, in_maps, core_ids=list(range(N_CORES)), trace=trace)
            return _assemble(res.results), res
        except Exception as err:  # transient NRT device errors
            last_err = err
    raise last_err


def kernel(x, wq, wk, wv, rel_h, rel_w):
    out, _ = run(
        dict(x=x, wq=wq, wk=wk, wv=wv, rel_h=rel_h, rel_w=rel_w),
        trace=bool(os.environ.get("ATTNCONV_TRACE")),
    )
    return out


# revision 22
# speedup vs baseline: 1.1855x; 1.0004x over previous
"""AttentionConv (sparse local attention, 7x7 window, per-channel softmax)
Trainium2 Bass kernel, SPMD across 8 NeuronCores.

Sharding: core i handles batch b = i//2 and channel half cg = i%2
(channels are independent through the whole op: 1x1 convs produce each
output channel from all input channels, and the softmax is per-channel
over the 7x7 window).

The relative-position bias for channels [0,128) is rel_h[u] (window row)
and for channels [128,256) is rel_w[v] (window col). To keep one SPMD
program for all cores, cg=1 cores receive spatially TRANSPOSED x (H<->W)
and their output is transposed back on the host; under that transpose
rel_w becomes a window-row bias, identical in structure to cg=0.

Per-core pipeline (fp16 score path, bf16 value path, f32 accumulate):
  1. PE GEMMs: q,k,v = W @ x in fp16 (negligible rounding vs fp32 for
     this data, half the DMA bytes, 16-bit matmul speed). K=256
     contraction in 2 chunks, N chunks of 512 (one PSUM bank each).
  2. k,v scattered into zero-padded 38x40 planes; each plane stored
     twice (interior at col 3 and col 2) so windowed reads for even AND
     odd window-cols are 4-byte aligned -> DVE 16-bit 2x perf mode.
  3. 7 bias-added copies of each padded k plane (bias for a fixed
     window-row is a per-partition scalar -> tensor_scalar 4x mode),
     trimmed to the 32 rows each window-row actually reads.
  4. main loop over window col v (7 iters), u-dim split in (4,3) halves
     for pipelining, diagonal access patterns covering all u at once:
       s = q * k_biased[window]  fp16     (DVE TT, 2x)
       e = exp(s) -> bf16                 (ScalarE ACT, unnormalized --
                                           scores are far inside exp's
                                           f32/bf16 range, so no
                                           max-subtraction pass needed)
       m = e * v[window]  bf16            (DVE TT, 2x)
       num += I @ m ; den += I @ e        (TensorE identity matmuls
                                           accumulating in PSUM f32;
                                           the otherwise-idle PE does
                                           all the j-summation work)
  5. out = num * reciprocal_approx_fast(den); split-queue DMA out.
Engine budget per core: DVE ~62us (bottleneck: 2 multiplies per window
element at 2 elem/cyc/lane), ACT ~46us, PE ~52us, ~85us measured total.
Measured (same-process paired A/B, fast device state): 83.27us on both
reps, vs 83.8/84.4 without the per-chunk PSUM tiles and 85.3-86.7 for
the session-start baseline (85.4us). Wins: trimmed bias copies (1280 vs
1520 elems), v-plane memzero on the scalar queue (unserializes gpsimd
ahead of the k scatter), last-v den-before-m tail, and PER-CHUNK PSUM
accumulators (nps0/1, dps0/1) -- Tile deps are tile-granular, so with
single [128,1024] accumulators the ch0 reciprocal/outmul sat ~1.5us
waiting for ch1's matmuls (ew=1420 on the first outmul in the trace).

Negative results (verified on HW, mostly paired A/B):
 - scalar_tensor_tensor (fused bias+mul) has no 2x uop -> 1x (111us).
 - Denser schedules REGRESS ~20% per-op on ALL engines (power/DVFS
   throttling): full one-v-ahead DVE pipelining 98.7us reproducible;
   kbO-copy split around the first m is span-neutral (the Tile
   scheduler already pipelines across v boundaries).
 - PE warm-up fill delays the cold GEMMs (+2us); merged den+num
   matmuls ([2,512] PSUM out AP) fail the NCC ISA check; splitting the
   last out-DMA across two queues costs more than it saves (+0.15us);
   x-DMA over 3 queues neutral-to-negative (per-queue DMA ~85-125GB/s).
 - DMA cannot read PSUM (kills host-side-divide tail tricks).
 - Per-DMA input staging tiles and u-half-split kb tiles (both trying
   to generalize the per-chunk-PSUM dep win to the startup path) are
   neutral and -0.4us respectively: pre-loop latency does not
   propagate to the finish line.
 - Slicing the LAST v's second half into per-u singles (to shorten the
   final s->exp->m->num chain from 3072- to 1024-element latencies)
   loses 0.6-1.0us: six extra DVE dispatches (~200cyc each) plus three
   extra ACT dispatches cost more than the latency they save; even the
   milder (4,2,1) slab split loses 0.7-2us. The (4,3) op granularity is
   the measured optimum from every direction.
Beware: the device toggles between a fast (~84us) and slow (~100us)
state per process; only same-process paired A/B is trustworthy
(in-process rep spread ~300ns, cross-process +-1.5us).
"""

import os

import numpy as np
import ml_dtypes

K = 7
PAD = 3
H = W = 32
HW = H * W
B = 4
C = 256
RS = 40          # padded plane row stride (elements); even => alignment
PR = H + 2 * PAD  # 38 padded rows
PW = PR * RS     # padded plane size per partition
N_CORES = 8

_NC_CACHE = {}


def _build_nc():
    import concourse.bass as bass
    import concourse.tile as tile
    from concourse import mybir, bacc

    bf16 = mybir.dt.bfloat16
    f16 = mybir.dt.float16
    f32 = mybir.dt.float32

    nc = bacc.Bacc(None)
    x_ext = nc.dram_tensor("x", [2, 128, HW], f16, kind="ExternalInput")
    w_ext = nc.dram_tensor("w", [3, 2, 128, 128], f16, kind="ExternalInput")
    b_ext = nc.dram_tensor("bias", [128, K], f32, kind="ExternalInput")
    i_ext = nc.dram_tensor("ident", [128, 128], bf16, kind="ExternalInput")
    o_ext = nc.dram_tensor("out", [128, HW], f32, kind="ExternalOutput")

    with tile.TileContext(nc) as tc:
        with (
            tc.tile_pool(name="consts", bufs=1) as consts,
            tc.tile_pool(name="kv", bufs=1) as kv,
            tc.tile_pool(name="fin", bufs=1) as fin,
            tc.tile_pool(name="psa", bufs=1, space="PSUM") as psa,
            tc.tile_pool(name="gt", bufs=1) as gt,
            tc.tile_pool(name="psg", bufs=4, space="PSUM") as psg,
            tc.tile_pool(name="sp", bufs=3) as sp,
            tc.tile_pool(name="ep", bufs=3) as ep,
            tc.tile_pool(name="mp", bufs=3) as mp,
        ):
            # DMAs spread across engine queues so they don't serialize.
            xsb = gt.tile([128, 2, HW], f16)
            wsb = gt.tile([128, 3, 2, 128], f16)
            bsb = consts.tile([128, K], f32)
            isb = consts.tile([128, 128], bf16)
            # k-GEMM path first on both fast queues; x split into halves
            # so the first k matmuls' data lands early.
            nc.gpsimd.dma_start(out=isb[:], in_=i_ext[:])
            nc.sync.dma_start(out=wsb[:, 1, 0, :], in_=w_ext[1, 0])
            nc.scalar.dma_start(out=wsb[:, 1, 1, :], in_=w_ext[1, 1])
            # (q and v weights each as one DMA below)
            nc.sync.dma_start(out=xsb[:, 0, 0:512], in_=x_ext[0][:, 0:512])
            nc.scalar.dma_start(out=xsb[:, 1, 0:512], in_=x_ext[1][:, 0:512])
            nc.sync.dma_start(out=xsb[:, 0, 512:HW], in_=x_ext[0][:, 512:HW])
            nc.scalar.dma_start(out=xsb[:, 1, 512:HW], in_=x_ext[1][:, 512:HW])
            nc.sync.dma_start(out=bsb[:], in_=b_ext[:])
            nc.scalar.dma_start(out=wsb[:, 0, 1, :], in_=w_ext[0, 1])
            nc.sync.dma_start(out=wsb[:, 0, 0, :], in_=w_ext[0, 0])
            nc.scalar.dma_start(out=wsb[:, 2, 1, :], in_=w_ext[2, 1])
            nc.sync.dma_start(out=wsb[:, 2, 0, :], in_=w_ext[2, 0])

            # q plane (read via a 7-way broadcast AP in the main loop)
            qsb = kv.tile([128, H, W], f16)
            # padded k/v planes; E holds interior at col 3 (for even v
            # window reads), O at col 2 (odd v reads at offset v-1).
            kpE = gt.tile([128, PR, RS], f16)
            kpO = gt.tile([128, PR, RS], f16)
            vpE = kv.tile([128, PR, RS], bf16)
            vpO = kv.tile([128, PR, RS], bf16)
            kbE = kv.tile([128, K, PR, RS], f16)
            kbO = kv.tile([128, K, PR, RS], f16)

            # k-plane zero-init on gpsimd; v-planes on the scalar queue
            # after its DMAs (frees ~2.8us of serial gpsimd memset time
            # ahead of the k scatter).
            nc.gpsimd.memset(kpE[:], 0.0)
            nc.gpsimd.memset(kpO[:], 0.0)
            nc.scalar.memzero(vpE[:])
            nc.scalar.memzero(vpO[:])

            # per-chunk PSUM accumulators: tile-granular deps mean the
            # ch0 reciprocal/outmul/DMA no longer wait for ch1's matmuls
            nps_t = [psa.tile([128, 512], f32, name=f"nps{c}") for c in range(2)]
            dps_t = [psa.tile([128, 512], f32, name=f"dps{c}") for c in range(2)]

            # PE pipeline/HAM warm-up: dummy matmuls into nps, whose
            # content is discarded when the first start=True accumulation
            # clears has_written.
            for _ in range(3):
                nc.tensor.matmul(nps_t[0][:, 0:128], isb[:], isb[:],
                                 start=True, stop=True, skip_group_check=True)

            # ---- GEMMs: wi 0=q, 1=k, 2=v; N chunks of 512 px (16 rows)
            # k first: it gates the longest pre-loop chain (bias copies).
            for wi in (1, 0, 2):
                for ch in range(2):
                    ps = psg.tile([128, 16, 32], f32)
                    for ci in range(2):
                        nc.tensor.matmul(
                            ps[:],
                            wsb[:, wi, ci, :],
                            xsb[:, ci, ch * 512:(ch + 1) * 512],
                            start=(ci == 0),
                            stop=(ci == 1),
                        )
                    r0 = PAD + 16 * ch
                    if wi == 0:
                        nc.scalar.copy(qsb[:, 16 * ch:16 * ch + 16, :], ps[:])
                    elif wi == 1:
                        nc.vector.tensor_copy(kpE[:, r0:r0 + 16, 3:35], ps[:])
                        nc.scalar.copy(kpO[:, r0:r0 + 16, 2:34], ps[:])
                    else:
                        nc.scalar.copy(vpE[:, r0:r0 + 16, 3:35], ps[:])
                        nc.scalar.copy(vpO[:, r0:r0 + 16, 2:34], ps[:])

            # biased k copies: kb*[u] = kp* + bias[:, u] (per-partition),
            # trimmed to rows u..u+31 -- the only rows window-row u reads.
            for u in range(K):
                nc.vector.tensor_scalar_add(
                    kbE[:, u, u:u + H, :], kpE[:, u:u + H, :], bsb[:, u:u + 1])
            for u in range(K):
                nc.vector.tensor_scalar_add(
                    kbO[:, u, u:u + H, :], kpO[:, u:u + H, :], bsb[:, u:u + 1])

            def window_ap(t, base_off, u0, nu, u_step):
                full = t[:]
                return bass.AP(
                    tensor=full.tensor,
                    offset=full.offset + base_off + u0 * u_step,
                    ap=[full.ap[0], [u_step, nu], [RS, H], [1, W]],
                )

            def q_bcast(nu):
                full = qsb[:]
                return bass.AP(
                    tensor=full.tensor,
                    offset=full.offset,
                    ap=[full.ap[0], [0, nu], [W, H], [1, W]],
                )

            # u-dim halves for finer DVE->ACT->PE pipelining. Measured
            # optimum: full-width ops 89.8us, quarters 87.2us, (3,4)
            # order 88.2us, this (4,3) split 85.4us. GpSimd offloads of
            # any slab regress (its in-loop TT is 3-5x slower than DVE).
            HALVES = ((0, 4), (4, 3))

            # ---- main loop over window col v
            for v in range(K):
                par = v & 1
                kb = kbO if par else kbE
                vp = vpO if par else vpE
                off = v - par  # even

                first = v == 0
                last = v == K - 1
                mm_args = []
                e_half = {}
                for u0, nu in HALVES:
                    s = sp.tile([128, nu, H, W], f16, tag=f"s{u0}")
                    nc.vector.tensor_mul(
                        s[:], q_bcast(nu), window_ap(kb, off, u0, nu, PW + RS))
                    e = ep.tile([128, nu, H, W], bf16, tag=f"e{u0}")
                    nc.scalar.activation(
                        e[:], s[:], mybir.ActivationFunctionType.Exp)
                    e_half[u0] = e
                    if last:
                        # tail: den accumulation straight after exp so
                        # it runs during the remaining m muls and the
                        # reciprocal unblocks early; sorted by chunk.
                        for want_ch in range(2):
                            for du in range(nu):
                                u = u0 + du
                                nc.tensor.matmul(
                                    dps_t[want_ch][:], isb[:],
                                    e[:, du, 16 * want_ch:16 * want_ch + 16, :],
                                    start=False, stop=(u == K - 1),
                                    skip_group_check=True,
                                )
                    m = mp.tile([128, nu, H, W], bf16, tag=f"m{u0}")
                    nc.vector.tensor_mul(
                        m[:], e[:], window_ap(vp, off, u0, nu, RS))
                    for du in range(nu):
                        u = u0 + du
                        for ch in range(2):
                            csl = slice(ch * 512, (ch + 1) * 512)
                            rsl = slice(16 * ch, 16 * ch + 16)
                            mm_args.append((csl, rsl, e, m, du, u))

                # den first so the tail's reciprocal (which needs only
                # dps) unblocks before the last num matmuls retire.
                if last:
                    # den already emitted above; num sorted by chunk.
                    for want_ch in (0, 1):
                        for csl, rsl, te, tm, du, u in mm_args:
                            if csl.start != want_ch * 512:
                                continue
                            nc.tensor.matmul(
                                nps_t[want_ch][:], isb[:], tm[:, du, rsl, :],
                                start=False, stop=(u == K - 1),
                                skip_group_check=True,
                            )
                else:
                    for csl, rsl, te, tm, du, u in mm_args:
                        ch = csl.start // 512
                        for kind in ("den", "num"):
                            t = te if kind == "den" else tm
                            acc = dps_t[ch] if kind == "den" else nps_t[ch]
                            nc.tensor.matmul(
                                acc[:], isb[:], t[:, du, rsl, :],
                                start=(first and u == 0),
                                stop=False,
                                skip_group_check=True,
                            )

            # ---- normalize and store: both reciprocals first (they
            # overlap the PE's final num matmuls), then multiply + DMA
            # per half on separate queues.
            rden = fin.tile([128, HW], f32)
            outsb = fin.tile([128, HW], f32)
            for ch in (0, 1):
                csl = slice(ch * 512, (ch + 1) * 512)
                nc.vector.reciprocal_approx_fast(
                    out=rden[:, csl], in_=dps_t[ch][:])
            for ch, eng in ((0, nc.sync), (1, nc.scalar)):
                csl = slice(ch * 512, (ch + 1) * 512)
                nc.vector.tensor_mul(
                    outsb[:, csl], nps_t[ch][:], rden[:, csl])
                eng.dma_start(out=o_ext[:, csl], in_=outsb[:, csl])

    nc.finalize()
    return nc


def _get_nc():
    if "nc" not in _NC_CACHE:
        _NC_CACHE["nc"] = _build_nc()
    return _NC_CACHE["nc"]


def _prep_in_maps(x, wq, wk, wv, rel_h, rel_w):
    bf = ml_dtypes.bfloat16
    ident = np.eye(128, dtype=bf)
    in_maps = []
    for core in range(N_CORES):
        b, cg = divmod(core, 2)
        xb = np.asarray(x[b], dtype=np.float32)
        if cg == 1:
            xb = xb.transpose(0, 2, 1)
        xb = np.ascontiguousarray(xb).reshape(2, 128, HW).astype(np.float16)
        rows = slice(cg * 128, (cg + 1) * 128)
        wt = np.stack([np.asarray(wq)[rows], np.asarray(wk)[rows],
                       np.asarray(wv)[rows]])          # [3, 128, 256]
        wt = np.ascontiguousarray(
            wt.transpose(0, 2, 1).astype(np.float16)).reshape(
            3, 2, 128, 128)                            # [wi, ci_chunk, ci, co]
        bias = np.ascontiguousarray(
            np.asarray(rel_h if cg == 0 else rel_w, dtype=np.float32))
        in_maps.append({"x": xb, "w": wt, "bias": bias, "ident": ident})
    return in_maps


def _assemble(results):
    out = np.empty((B, C, H, W), np.float32)
    for core in range(N_CORES):
        b, cg = divmod(core, 2)
        o = results[core]["out"].reshape(128, H, W)
        if cg == 1:
            o = o.transpose(0, 2, 1)
        out[b, cg * 128:(cg + 1) * 128] = o
    return out


def run(inputs, trace=False):
    """Returns (output, BassKernelResults)."""
    from concourse import bass_utils

    nc = _get_nc()
    in_maps = _prep_in_maps(**inputs)
    last_err = None
    for _attempt in range(3):
        try:
            res = bass_utils.run_bass_kernel_spmd(
                nc, in_maps, core_ids=list(range(N_CORES)), trace=trace)
            return _assemble(res.results), res
        except Exception as err:  # transient NRT device errors
            last_err = err
    raise last_err


def kernel(x, wq, wk, wv, rel_h, rel_w):
    out, _ = run(
        dict(x=x, wq=wq, wk=wk, wv=wv, rel_h=rel_h, rel_w=rel_w),
        trace=bool(os.environ.get("ATTNCONV_TRACE")),
    )
    return out
